# revision 12
# baseline (speedup 1.0000x reference)
"""Trainium2 Bass kernel for nn_CombinedLoss (L1 + 0.5*SSIM + 0.1*Wavelet).

Sharding: pure data-parallel over batch (32 images -> 4 per core x 8 cores).
Each core returns a [1, 64] f32 vector of partial sums; host combines.

On-chip plan per core (4 images, 512x512, bf16 data / f32 PSUM):
  - stage-in: DMA-cast f32->bf16, p^2/t^2 on ACT, 2pt via STT, L1 |p-t| reduce
  - pass A (PE): row-direction conv for {p, t, p^2+t^2, 2pt} + Haar row-pairs
    for {p, t}, via "data-form" matmuls (lhsT = image blocks, rhs = packed
    banded Gaussian Gp / pair matrix W1p). Output comes out transposed.
  - pass B (PE): column-direction conv / Haar col-pairs on the transposed
    intermediates -> full conv fields mu1, mu2, S2=conv(p^2+t^2), D2=2conv(pt)
    and DWT level-1 quadrants, directly in natural orientation.
  - SSIM map: scalar_tensor_tensor chain with folded constants,
    reciprocal_approx_fast for the division, accum_out for the sum.
  - Wavelet levels 2,3: same two-pass machinery on the cA quadrant.
    Soft-threshold via ACT relus, diffs via STT, sum via reduce(abs).
"""

import sys

sys.path.insert(0, "/opt/trn_rl_repo")

import numpy as np

import concourse.bass as bass
import concourse.bacc as bacc
import concourse.mybir as mybir
from concourse.tile import TileContext

F32 = mybir.dt.float32
BF16 = mybir.dt.bfloat16
ALU = mybir.AluOpType
ACTF = mybir.ActivationFunctionType

P = 128
H = W = 512
NIMG = 4          # images per core
NCORES = 8
WIN = 11
SIGMA = 1.5
C1 = 0.01 ** 2
C2 = 0.03 ** 2
C12 = C1 + C2
GW = 138          # padded conv band window width (128 + 2*5)

# wavelet thresholds: my level L (1=finest 256^2 bands) maps to reference
# level_idx (1=coarsest): ref_idx = 4 - L
T_LVL = {1: (50.0 / 4.0) / 255.0, 2: (50.0 / 2.0) / 255.0, 3: 50.0 / 255.0}

# accumulator column map (acc is [128, 64] f32; out = ones^T @ acc -> [1,64])
COL_L1 = 0        # + img               (4)
COL_SSIM = 4      # + 4*img + m         (16)
COL_W1 = 20       # + 4*img + m         (16)
COL_W2 = 36       # + 2*img + m2        (8)
COL_W3 = 44       # + 2*img + {0,1}     (8)
NACC = 64


def _np_bf16():
    return mybir.dt.np(BF16)


def _gauss_taps():
    """11 Gaussian taps, bf16-quantized with the quantization residual
    redistributed so the bf16 tap-sum matches the f32 tap-sum (a tap-sum
    error gamma biases sigma12 by -2*gamma*mu1*mu2, which is large relative
    to the tiny ssim_map mean)."""
    x = np.arange(WIN, dtype=np.float32) - WIN // 2
    g32 = np.exp(-(x ** 2) / (2.0 * np.float32(SIGMA) ** 2))
    g32 = g32 / g32.sum()
    bf = _np_bf16()
    gb = g32.astype(bf)
    target = g32.astype(np.float64).sum()
    for _ in range(40):
        gamma = gb.astype(np.float64).sum() - target
        if abs(gamma) < 1e-7:
            break
        best = None
        for i in range(WIN):
            v = gb[i]
            hi = np.asarray(10.0, dtype=bf)
            lo = np.asarray(-10.0, dtype=bf)
            for cand in (np.nextafter(v, hi, dtype=bf),
                         np.nextafter(v, lo, dtype=bf)):
                g2 = gb.copy()
                g2[i] = cand
                newg = abs(g2.astype(np.float64).sum() - target)
                drift = abs(float(cand) - g32[i]) / g32[i]
                if newg < abs(gamma) and drift < 0.01 and (
                        best is None or newg < best[0]):
                    best = (newg, i, cand)
        if best is None:
            break
        gb[best[1]] = best[2]
    return gb.astype(np.float64)


def _build_consts():
    """Packed conv band Gp [512,138], Haar row W1p [512,128] (+-1),
    Haar col Wcp [512,128] (+-0.5)."""
    g = _gauss_taps()
    G = np.zeros((512, 512), dtype=np.float64)
    for h in range(512):
        for j in range(WIN):
            hp = h + j - WIN // 2
            if 0 <= hp < 512:
                G[h, hp] = g[j]
    Gp = np.zeros((512, GW), dtype=np.float64)
    for k in range(4):
        a = min(max(128 * k - 5, 0), 512 - GW)
        Gp[128 * k:128 * k + 128, :] = G[128 * k:128 * k + 128, a:a + GW]
    W1p = np.zeros((512, 128), dtype=np.float64)
    Wcp = np.zeros((512, 128), dtype=np.float64)
    for k in range(4):
        for j in range(64):
            r0 = 128 * k + 2 * j
            W1p[r0, j] = 1.0
            W1p[r0 + 1, j] = 1.0
            W1p[r0, 64 + j] = 1.0
            W1p[r0 + 1, 64 + j] = -1.0
            Wcp[r0, j] = 0.5
            Wcp[r0 + 1, j] = 0.5
            Wcp[r0, 64 + j] = 0.5
            Wcp[r0 + 1, 64 + j] = -0.5
    bf = _np_bf16()
    return Gp.astype(bf), W1p.astype(bf), Wcp.astype(bf)


def _conv_out_off(k):
    return min(max(128 * k - 5, 0), 512 - GW)


def _register_consts(nc, values, dtype=F32):
    for v in values:
        v = float(v)
        if (dtype, v) in nc.const_aps.aps:
            continue
        t = nc.alloc_sbuf_tensor(f"const-{dtype.name}-{v}", [128, 1], dtype)
        nc.gpsimd.memset(t.ap(), v)
        nc.const_aps.aps[(dtype, v)] = t.ap()
    nc.all_engine_barrier()


def build_nc():
    nc = bacc.Bacc()
    _register_consts(nc, [-T_LVL[1], -T_LVL[2], -T_LVL[3]])

    pred_d = nc.dram_tensor("pred", [NIMG, H, W], F32, kind="ExternalInput")
    targ_d = nc.dram_tensor("target", [NIMG, H, W], F32, kind="ExternalInput")
    gp_d = nc.dram_tensor("gp", [512, GW], BF16, kind="ExternalInput")
    gp2_d = nc.dram_tensor("gp2", [512, GW], BF16, kind="ExternalInput")
    w1p_d = nc.dram_tensor("w1p", [512, 128], BF16, kind="ExternalInput")
    wcp_d = nc.dram_tensor("wcp", [512, 128], BF16, kind="ExternalInput")
    out_d = nc.dram_tensor("out", [1, NACC], F32, kind="ExternalOutput")

    with TileContext(nc) as tc:
        with (
            tc.tile_pool(name="const", bufs=1) as cpool,
            tc.tile_pool(name="img", bufs=2) as ipool,
            tc.tile_pool(name="mid", bufs=2) as mpool,
            tc.tile_pool(name="tmp", bufs=3) as tpool,
            tc.tile_pool(name="wav", bufs=2) as wpool,
            tc.tile_pool(name="psum", bufs=1, space="PSUM") as pspool,
        ):
            # ---- constants ----
            gp = cpool.tile([P, 4, GW], BF16, tag="gp")
            gp2 = cpool.tile([P, 4, GW], BF16, tag="gp2")
            w1p = cpool.tile([P, 4, 128], BF16, tag="w1p")
            wcp = cpool.tile([P, 4, 128], BF16, tag="wcp")
            nc.sync.dma_start(gp[:], gp_d.rearrange("(c p) n -> p c n", p=P))
            nc.sync.dma_start(gp2[:], gp2_d.rearrange("(c p) n -> p c n", p=P))
            nc.sync.dma_start(w1p[:], w1p_d.rearrange("(c p) n -> p c n", p=P))
            nc.sync.dma_start(wcp[:], wcp_d.rearrange("(c p) n -> p c n", p=P))

            gpc = cpool.tile([P, 4, GW], BF16, tag="gpc")
            gp2c = cpool.tile([P, 4, GW], BF16, tag="gp2c")
            w1pc = cpool.tile([P, 4, 128], BF16, tag="w1pc")
            wcpc = cpool.tile([P, 4, 128], BF16, tag="wcpc")
            nc.vector.tensor_copy(gpc[:], gp[:])
            nc.vector.tensor_copy(gp2c[:], gp2[:])
            nc.vector.tensor_copy(w1pc[:], w1p[:])
            nc.vector.tensor_copy(wcpc[:], wcp[:])
            gp, gp2, w1p, wcp = gpc, gp2c, w1pc, wcpc

            acc = cpool.tile([P, NACC], F32, tag="acc")
            nc.vector.memset(acc[:], 0.0)
            ones = cpool.tile([P, 1], F32, tag="ones")
            nc.vector.memset(ones[:], 1.0)

            for i in range(NIMG):
                # ---- stage-in ----
                p_t = ipool.tile([P, 4, W], BF16, tag="p")
                t_t = ipool.tile([P, 4, W], BF16, tag="t")
                nc.gpsimd.dma_start(
                    p_t[:], pred_d[i].rearrange("(c p) w -> p c w", p=P))
                nc.gpsimd.dma_start(
                    t_t[:], targ_d[i].rearrange("(c p) w -> p c w", p=P))

                p2_t = ipool.tile([P, 4, W], BF16, tag="p2")
                t2_t = ipool.tile([P, 4, W], BF16, tag="t2")
                pt2_t = ipool.tile([P, 4, W], BF16, tag="pt2")
                q_t = ipool.tile([P, 4, W], BF16, tag="q")
                nc.scalar.activation(p2_t[:], p_t[:], ACTF.Square)
                nc.scalar.activation(t2_t[:], t_t[:], ACTF.Square)
                # p*t (x2 folded into gp2 conv weights); tiny pre-touches
                # absorb the two DMA waits one at a time on gpsimd
                preg = tpool.tile([P, 2], BF16, tag="preg")
                nc.gpsimd.tensor_copy(preg[:, 0:1], p_t[:, 0, 0:1])
                nc.gpsimd.tensor_copy(preg[:, 1:2], t_t[:, 0, 0:1])
                nc.gpsimd.tensor_mul(pt2_t[:], p_t[:], t_t[:])
                # tiny pre-touches: absorb the two DMA waits on DVE one
                # at a time (DVE instr structs have few sync-wait slots)
                pre = tpool.tile([P, 2], BF16, tag="pre")
                nc.vector.tensor_copy(pre[:, 0:1], p_t[:, 0, 0:1])
                nc.vector.tensor_copy(pre[:, 1:2], t_t[:, 0, 0:1])
                # |p - t| -> L1 partial
                nc.vector.tensor_sub(q_t[:], p_t[:], t_t[:])
                nc.vector.tensor_reduce(
                    acc[:, COL_L1 + i:COL_L1 + i + 1], q_t[:],
                    axis=mybir.AxisListType.XY, op=ALU.add,
                    apply_absolute_value=True)

                # ---- pass A: row conv + row pairs (data-form matmuls) ----
                rp = mpool.tile([P, 4, W], BF16, tag="rp")
                rt = mpool.tile([P, 4, W], BF16, tag="rt")
                rS = mpool.tile([P, 4, W], BF16, tag="rS")
                rD = mpool.tile([P, 4, W], BF16, tag="rD")
                rwp = mpool.tile([P, 4, W], BF16, tag="rwp")
                rwt = mpool.tile([P, 4, W], BF16, tag="rwt")
                for m in range(4):
                    bP = pspool.tile([P, W], F32, tag="ps0")
                    bT = pspool.tile([P, W], F32, tag="ps1")
                    bS = pspool.tile([P, W], F32, tag="ps2")
                    bD = pspool.tile([P, W], F32, tag="ps3")
                    bWp = pspool.tile([P, W], F32, tag="ps4")
                    bWt = pspool.tile([P, W], F32, tag="ps5")
                    for k in range(4):
                        a = _conv_out_off(k)
                        st = k == 0
                        mm = nc.tensor.matmul
                        pb = p_t[:, k, 128 * m:128 * m + 128]
                        tb = t_t[:, k, 128 * m:128 * m + 128]
                        gw = gp[:, k, :]
                        mm(bP[:, a:a + GW], pb, gw, start=st, stop=k == 3)
                        mm(bT[:, a:a + GW], tb, gw, start=st, stop=k == 3)
                        mm(bS[:, a:a + GW], p2_t[:, k, 128 * m:128 * m + 128],
                           gw, start=st, stop=False)
                        mm(bS[:, a:a + GW], t2_t[:, k, 128 * m:128 * m + 128],
                           gw, start=False, stop=k == 3)
                        mm(bD[:, a:a + GW], pt2_t[:, k, 128 * m:128 * m + 128],
                           gp2[:, k, :], start=st, stop=k == 3)
                        # Haar row pairs: RS cols [64k,64k+64), RD [256+64k,..)
                        mm(bWp[:, 64 * k:64 * k + 64], pb, w1p[:, k, 0:64],
                           start=st, stop=False)
                        mm(bWp[:, 256 + 64 * k:256 + 64 * k + 64], pb,
                           w1p[:, k, 64:128], start=False, stop=k == 3)
                        mm(bWt[:, 64 * k:64 * k + 64], tb, w1p[:, k, 0:64],
                           start=st, stop=False)
                        mm(bWt[:, 256 + 64 * k:256 + 64 * k + 64], tb,
                           w1p[:, k, 64:128], start=False, stop=k == 3)
                    nc.scalar.copy(rp[:, m, :], bP[:])
                    nc.scalar.copy(rt[:, m, :], bT[:])
                    nc.vector.tensor_copy(rS[:, m, :], bS[:])
                    nc.vector.tensor_copy(rD[:, m, :], bD[:])
                    nc.scalar.copy(rwp[:, m, :], bWp[:])
                    nc.vector.tensor_copy(rwt[:, m, :], bWt[:])

                # ---- pass B: col conv + col pairs; fused SSIM / wavelet ----
                cAp = wpool.tile([P, 2, 256], BF16, tag="cAp")
                cAt = wpool.tile([P, 2, 256], BF16, tag="cAt")
                for m in range(4):
                    bM1 = pspool.tile([P, W], F32, tag="ps0")
                    bM2 = pspool.tile([P, W], F32, tag="ps1")
                    bS2 = pspool.tile([P, W], F32, tag="ps2")
                    bD2 = pspool.tile([P, W], F32, tag="ps3")
                    bQp = pspool.tile([P, W], F32, tag="ps4")
                    bQt = pspool.tile([P, W], F32, tag="ps5")
                    for k in range(4):
                        a = _conv_out_off(k)
                        st = k == 0
                        mm = nc.tensor.matmul
                        gw = gp[:, k, :]
                        mm(bM1[:, a:a + GW], rp[:, k, 128 * m:128 * m + 128],
                           gw, start=st, stop=k == 3)
                        mm(bM2[:, a:a + GW], rt[:, k, 128 * m:128 * m + 128],
                           gw, start=st, stop=k == 3)
                        mm(bS2[:, a:a + GW], rS[:, k, 128 * m:128 * m + 128],
                           gw, start=st, stop=k == 3)
                        mm(bD2[:, a:a + GW], rD[:, k, 128 * m:128 * m + 128],
                           gw, start=st, stop=k == 3)
                        mm(bQp[:, 64 * k:64 * k + 64],
                           rwp[:, k, 128 * m:128 * m + 128],
                           wcp[:, k, 0:64], start=st, stop=False)
                        mm(bQp[:, 256 + 64 * k:256 + 64 * k + 64],
                           rwp[:, k, 128 * m:128 * m + 128],
                           wcp[:, k, 64:128], start=False, stop=k == 3)
                        mm(bQt[:, 64 * k:64 * k + 64],
                           rwt[:, k, 128 * m:128 * m + 128],
                           wcp[:, k, 0:64], start=st, stop=False)
                        mm(bQt[:, 256 + 64 * k:256 + 64 * k + 64],
                           rwt[:, k, 128 * m:128 * m + 128],
                           wcp[:, k, 64:128], start=False, stop=k == 3)

                    # SSIM chain on this [128, 512] chunk
                    m1s = tpool.tile([P, W], BF16, tag="m1s")
                    sq1 = tpool.tile([P, W], BF16, tag="sq1")
                    sq2 = tpool.tile([P, W], BF16, tag="sq2")
                    n1p = tpool.tile([P, W], BF16, tag="n1p")
                    d1 = tpool.tile([P, W], BF16, tag="d1")
                    n2 = tpool.tile([P, W], BF16, tag="n2")
                    d2 = tpool.tile([P, W], BF16, tag="d2")
                    num = tpool.tile([P, W], BF16, tag="num")
                    den = tpool.tile([P, W], F32, tag="den")
                    rcp = tpool.tile([P, W], F32, tag="rcp")
                    sst = tpool.tile([P, W], BF16, tag="sst")
                    nc.vector.tensor_copy(m1s[:], bM1[:])
                    nc.scalar.activation(sq1[:], bM1[:], ACTF.Square)
                    nc.scalar.activation(sq2[:], bM2[:], ACTF.Square)
                    stt = nc.vector.scalar_tensor_tensor
                    # n1p = 2*mu1*mu2
                    stt(n1p[:], bM2[:], 2.0, m1s[:], ALU.mult, ALU.mult)
                    # d1 = (sq1 + C1) + sq2
                    stt(d1[:], sq1[:], C1, sq2[:], ALU.add, ALU.add)
                    # n2 = (D2 + C2) - n1p
                    stt(n2[:], bD2[:], C2, n1p[:], ALU.add, ALU.subtract)
                    # d2 = (S2 + C1 + C2) - d1
                    stt(d2[:], bS2[:], C12, d1[:], ALU.add, ALU.subtract)
                    # num = (n1p + C1) * n2
                    stt(num[:], n1p[:], C1, n2[:], ALU.add, ALU.mult)
                    # den = d1 * d2 (f32 for reciprocal)
                    stt(den[:], d1[:], 0.0, d2[:], ALU.bypass, ALU.mult)
                    nc.vector.reciprocal_approx_fast(rcp[:], den[:])
                    stt(sst[:], num[:], 0.0, rcp[:], ALU.bypass, ALU.mult,
                        accum_out=acc[:, COL_SSIM + 4 * i + m:
                                      COL_SSIM + 4 * i + m + 1])

                    # wavelet L1 quadrants of bQp/bQt
                    _wav_detail(nc, tpool, acc, COL_W1 + 4 * i + m,
                                bQp, bQt, m, cAp, cAt, T_LVL[1])

                # ---- wavelet level 2 on cA [256,256] ----
                cA2p, cA2t = _wav_level2(nc, tc, wpool, tpool, pspool,
                                         w1p, wcp, acc, i, cAp, cAt)
                # ---- wavelet level 3 on cA2 [128,128] ----
                _wav_level3(nc, wpool, tpool, pspool, w1p, wcp, acc, i,
                            cA2p, cA2t)

            # ---- final reduction: out = ones^T @ acc ----
            outp = pspool.tile([1, NACC], F32, tag="outp")
            nc.tensor.matmul(outp[:], ones[:], acc[:], start=True, stop=True)
            outs = cpool.tile([1, NACC], F32, tag="outs")
            nc.scalar.copy(outs[:], outp[:])
            nc.sync.dma_start(out_d[:], outs[:])

    nc.finalize()
    return nc


def _soft_diff_sum(nc, tpool, acc_col_ap, fp, ft, thr, tag):
    """acc_col += sum |soft(fp) - soft(ft)| over a detail field.

    fp/ft are PSUM (or SBUF) APs of identical shape [pp, n].
    soft(x) = relu(x - T) - relu(-x - T).
    """
    pp = fp.shape[0]
    n = int(np.prod(fp.shape[1:]))
    spp = tpool.tile([pp, n], BF16, tag="spp")
    spn = tpool.tile([pp, n], BF16, tag="spn")
    stp = tpool.tile([pp, n], BF16, tag="stp")
    stn = tpool.tile([pp, n], BF16, tag="stn")
    q1 = tpool.tile([pp, n], BF16, tag="wq1")
    q2 = tpool.tile([pp, n], BF16, tag="wq2")
    q3 = tpool.tile([pp, n], BF16, tag="wq3")
    act = nc.scalar.activation
    act(spp[:], fp, ACTF.Relu, bias=-thr, scale=1.0)
    act(spn[:], fp, ACTF.Relu, bias=-thr, scale=-1.0)
    act(stp[:], ft, ACTF.Relu, bias=-thr, scale=1.0)
    act(stn[:], ft, ACTF.Relu, bias=-thr, scale=-1.0)
    nc.vector.scalar_tensor_tensor(
        q1[:], spp[:], 0.0, stp[:], ALU.bypass, ALU.subtract)
    nc.gpsimd.tensor_sub(q2[:], spn[:], stn[:])
    nc.vector.scalar_tensor_tensor(
        q3[:], q1[:], 0.0, q2[:], ALU.bypass, ALU.subtract)
    nc.vector.tensor_reduce(
        acc_col_ap, q3[:], axis=mybir.AxisListType.X, op=ALU.add,
        apply_absolute_value=True)


def _wav_detail(nc, tpool, acc, col, bQp, bQt, m, cAp, cAt, thr):
    """Handle one [128,512] chunk of the level-1 DWT output.

    m in {0,1}: rows are RS -> cols [0,256)=cA (save), [256,512)=cV (detail).
    m in {2,3}: rows are RD -> cH | cD, both detail.
    """
    if m < 2:
        nc.scalar.copy(cAp[:, m, :], bQp[:, 0:256])
        nc.scalar.copy(cAt[:, m, :], bQt[:, 0:256])
        _soft_diff_sum(nc, tpool, acc[:, col:col + 1],
                       bQp[:, 256:512], bQt[:, 256:512], thr, "a")
    else:
        _soft_diff_sum(nc, tpool, acc[:, col:col + 1],
                       bQp[:], bQt[:], thr, "b")


def _wav_level2(nc, tc, wpool, tpool, pspool, w1p, wcp, acc, i, cAp, cAt):
    """Level-2 DWT on cA [256,256] (stored [128, 2, 256])."""
    rw2p = wpool.tile([P, 2, 256], BF16, tag="rw2p")
    rw2t = wpool.tile([P, 2, 256], BF16, tag="rw2t")
    for m in range(2):
        b2p = pspool.tile([P, 256], F32, tag="ps0")
        b2t = pspool.tile([P, 256], F32, tag="ps1")
        for k in range(2):
            st = k == 0
            mm = nc.tensor.matmul
            mm(b2p[:, 64 * k:64 * k + 64],
               cAp[:, k, 128 * m:128 * m + 128], w1p[:, k, 0:64],
               start=st, stop=False)
            mm(b2p[:, 128 + 64 * k:128 + 64 * k + 64],
               cAp[:, k, 128 * m:128 * m + 128], w1p[:, k, 64:128],
               start=False, stop=k == 1)
            mm(b2t[:, 64 * k:64 * k + 64],
               cAt[:, k, 128 * m:128 * m + 128], w1p[:, k, 0:64],
               start=st, stop=False)
            mm(b2t[:, 128 + 64 * k:128 + 64 * k + 64],
               cAt[:, k, 128 * m:128 * m + 128], w1p[:, k, 64:128],
               start=False, stop=k == 1)
        nc.scalar.copy(rw2p[:, m, :], b2p[:])
        nc.vector.tensor_copy(rw2t[:, m, :], b2t[:])

    cA2p = wpool.tile([P, 128], BF16, tag="cA2p")
    cA2t = wpool.tile([P, 128], BF16, tag="cA2t")
    for m in range(2):
        d2p = pspool.tile([P, 256], F32, tag="ps2")
        d2t = pspool.tile([P, 256], F32, tag="ps3")
        for k in range(2):
            st = k == 0
            mm = nc.tensor.matmul
            mm(d2p[:, 64 * k:64 * k + 64],
               rw2p[:, k, 128 * m:128 * m + 128], wcp[:, k, 0:64],
               start=st, stop=False)
            mm(d2p[:, 128 + 64 * k:128 + 64 * k + 64],
               rw2p[:, k, 128 * m:128 * m + 128], wcp[:, k, 64:128],
               start=False, stop=k == 1)
            mm(d2t[:, 64 * k:64 * k + 64],
               rw2t[:, k, 128 * m:128 * m + 128], wcp[:, k, 0:64],
               start=st, stop=False)
            mm(d2t[:, 128 + 64 * k:128 + 64 * k + 64],
               rw2t[:, k, 128 * m:128 * m + 128], wcp[:, k, 64:128],
               start=False, stop=k == 1)
        col = COL_W2 + 2 * i + m
        if m == 0:
            nc.scalar.copy(cA2p[:], d2p[:, 0:128])
            nc.scalar.copy(cA2t[:], d2t[:, 0:128])
            _soft_diff_sum(nc, tpool, acc[:, col:col + 1],
                           d2p[:, 128:256], d2t[:, 128:256], T_LVL[2], "c")
        else:
            _soft_diff_sum(nc, tpool, acc[:, col:col + 1],
                           d2p[:], d2t[:], T_LVL[2], "d")
    return cA2p, cA2t


def _wav_level3(nc, wpool, tpool, pspool, w1p, wcp, acc, i, cA2p, cA2t):
    """Level-3 DWT on cA2 [128,128]."""
    rw3p = wpool.tile([P, 128], BF16, tag="rw3p")
    rw3t = wpool.tile([P, 128], BF16, tag="rw3t")
    b3p = pspool.tile([P, 128], F32, tag="ps0")
    b3t = pspool.tile([P, 128], F32, tag="ps1")
    mm = nc.tensor.matmul
    mm(b3p[:, 0:64], cA2p[:], w1p[:, 0, 0:64], start=True, stop=False)
    mm(b3p[:, 64:128], cA2p[:], w1p[:, 0, 64:128], start=False, stop=True)
    mm(b3t[:, 0:64], cA2t[:], w1p[:, 0, 0:64], start=True, stop=False)
    mm(b3t[:, 64:128], cA2t[:], w1p[:, 0, 64:128], start=False, stop=True)
    nc.scalar.copy(rw3p[:], b3p[:])
    nc.vector.tensor_copy(rw3t[:], b3t[:])

    d3p = pspool.tile([P, 128], F32, tag="ps2")
    d3t = pspool.tile([P, 128], F32, tag="ps3")
    mm(d3p[:, 0:64], rw3p[:], wcp[:, 0, 0:64], start=True, stop=False)
    mm(d3p[:, 64:128], rw3p[:], wcp[:, 0, 64:128], start=False, stop=True)
    mm(d3t[:, 0:64], rw3t[:], wcp[:, 0, 0:64], start=True, stop=False)
    mm(d3t[:, 64:128], rw3t[:], wcp[:, 0, 64:128], start=False, stop=True)
    # quadrants: partitions 0:64 = RS rows (cA3 | cV3), 64:128 = RD (cH3|cD3)
    # detail fields: cV3 = [0:64, 64:128], cH3+cD3 = [64:128, 0:128]
    col = COL_W3 + 2 * i
    _soft_diff_sum(nc, tpool, acc[0:64, col:col + 1],
                   d3p[0:64, 64:128], d3t[0:64, 64:128], T_LVL[3], "e")
    _soft_diff_sum(nc, tpool, acc[64:128, col + 1:col + 2],
                   d3p[64:128, 0:128], d3t[64:128, 0:128], T_LVL[3], "f")


_NC_CACHE = None


def _get_nc():
    global _NC_CACHE
    if _NC_CACHE is None:
        _NC_CACHE = build_nc()
    return _NC_CACHE


def kernel(pred: np.ndarray, target: np.ndarray) -> np.ndarray:
    from concourse.bass_utils import run_bass_kernel_spmd

    pred = np.ascontiguousarray(np.asarray(pred, dtype=np.float32)
                                .reshape(32, H, W))
    target = np.ascontiguousarray(np.asarray(target, dtype=np.float32)
                                  .reshape(32, H, W))
    gp, w1p, wcp = _build_consts()
    gp2 = (gp.astype(np.float32) * 2.0).astype(_np_bf16())

    in_maps = []
    for c in range(NCORES):
        in_maps.append({
            "pred": np.ascontiguousarray(pred[NIMG * c:NIMG * (c + 1)]),
            "target": np.ascontiguousarray(target[NIMG * c:NIMG * (c + 1)]),
            "gp": gp, "gp2": gp2, "w1p": w1p, "wcp": wcp,
        })

    nc = _get_nc()
    res = run_bass_kernel_spmd(nc, in_maps, core_ids=list(range(NCORES)))
    partials = np.stack([r["out"][0].astype(np.float64)
                         for r in res.results])  # [8, 64]
    tot = partials.sum(axis=0)

    npix = 32.0 * H * W
    l1 = tot[COL_L1:COL_L1 + 4].sum() / npix
    ssim_mean = tot[COL_SSIM:COL_SSIM + 16].sum() / npix
    ssim_loss = np.clip(1.0 - ssim_mean, 0.0, 2.0)
    w1 = tot[COL_W1:COL_W1 + 16].sum()   # finest: 256^2 bands
    w2 = tot[COL_W2:COL_W2 + 8].sum()    # 128^2 bands
    w3 = tot[COL_W3:COL_W3 + 8].sum()    # coarsest: 64^2 bands
    wav = (
        (w3 / (32.0 * 64 * 64) / 3.0) / 1.0
        + (w2 / (32.0 * 128 * 128) / 3.0) / 2.0
        + (w1 / (32.0 * 256 * 256) / 3.0) / 3.0
    )
    loss = l1 + 0.5 * ssim_loss + 0.1 * wav
    return np.float32(loss)


# revision 22
# speedup vs baseline: 42.2854x; 42.2854x over previous
"""Trainium2 Bass kernel for nn_CombinedLoss (L1 + 0.5*SSIM + 0.1*Wavelet).

Sharding: pure data-parallel over batch (32 images -> 4 per core x 8 cores).
Each core returns a [1, 64] f32 vector of partial sums; host combines.

On-chip plan per core (4 images, 512x512, bf16 data / f32 PSUM):
  - stage-in: DMA-cast f32->bf16, p^2/t^2 on ACT, 2pt via STT, L1 |p-t| reduce
  - pass A (PE): row-direction conv for {p, t, p^2+t^2, 2pt} + Haar row-pairs
    for {p, t}, via "data-form" matmuls (lhsT = image blocks, rhs = packed
    banded Gaussian Gp / pair matrix W1p). Output comes out transposed.
  - pass B (PE): column-direction conv / Haar col-pairs on the transposed
    intermediates -> full conv fields mu1, mu2, S2=conv(p^2+t^2), D2=2conv(pt)
    and DWT level-1 quadrants, directly in natural orientation.
  - SSIM map: scalar_tensor_tensor chain with folded constants,
    reciprocal_approx_fast for the division, accum_out for the sum.
  - Wavelet levels 2,3: same two-pass machinery on the cA quadrant.
    Soft-threshold via ACT relus, diffs via STT, sum via reduce(abs).
"""

import sys

sys.path.insert(0, "/opt/trn_rl_repo")

import numpy as np

import concourse.bass as bass
import concourse.bacc as bacc
import concourse.mybir as mybir
from concourse.tile import TileContext

F32 = mybir.dt.float32
BF16 = mybir.dt.bfloat16
ALU = mybir.AluOpType
ACTF = mybir.ActivationFunctionType

P = 128
H = W = 512
NIMG = 4          # images per core
NCORES = 8
WIN = 11
SIGMA = 1.5
C1 = 0.01 ** 2
C2 = 0.03 ** 2
C12 = C1 + C2
GW = 138          # padded conv band window width (128 + 2*5)

# wavelet thresholds: my level L (1=finest 256^2 bands) maps to reference
# level_idx (1=coarsest): ref_idx = 4 - L
T_LVL = {1: (50.0 / 4.0) / 255.0, 2: (50.0 / 2.0) / 255.0, 3: 50.0 / 255.0}

# accumulator column map (acc is [128, 64] f32; out = ones^T @ acc -> [1,64])
COL_L1 = 0        # + img               (4)
COL_SSIM = 4      # + 4*img + m         (16)
COL_W1 = 20       # + 4*img + m         (16)
COL_W2 = 36       # + 2*img + m2        (8)
COL_W3 = 44       # + 2*img + {0,1}     (8)
NACC = 64


def _np_bf16():
    return mybir.dt.np(BF16)


def _gauss_taps():
    """11 Gaussian taps, bf16-quantized with the quantization residual
    redistributed so the bf16 tap-sum matches the f32 tap-sum (a tap-sum
    error gamma biases sigma12 by -2*gamma*mu1*mu2, which is large relative
    to the tiny ssim_map mean)."""
    x = np.arange(WIN, dtype=np.float32) - WIN // 2
    g32 = np.exp(-(x ** 2) / (2.0 * np.float32(SIGMA) ** 2))
    g32 = g32 / g32.sum()
    bf = _np_bf16()
    gb = g32.astype(bf)
    target = g32.astype(np.float64).sum()
    for _ in range(40):
        gamma = gb.astype(np.float64).sum() - target
        if abs(gamma) < 1e-7:
            break
        best = None
        for i in range(WIN):
            v = gb[i]
            hi = np.asarray(10.0, dtype=bf)
            lo = np.asarray(-10.0, dtype=bf)
            for cand in (np.nextafter(v, hi, dtype=bf),
                         np.nextafter(v, lo, dtype=bf)):
                g2 = gb.copy()
                g2[i] = cand
                newg = abs(g2.astype(np.float64).sum() - target)
                drift = abs(float(cand) - g32[i]) / g32[i]
                if newg < abs(gamma) and drift < 0.01 and (
                        best is None or newg < best[0]):
                    best = (newg, i, cand)
        if best is None:
            break
        gb[best[1]] = best[2]
    return gb.astype(np.float64)


def _build_consts():
    """Packed conv band Gp [512,138], Haar row W1p [512,128] (+-1),
    Haar col Wcp [512,128] (+-0.5)."""
    g = _gauss_taps()
    G = np.zeros((512, 512), dtype=np.float64)
    for h in range(512):
        for j in range(WIN):
            hp = h + j - WIN // 2
            if 0 <= hp < 512:
                G[h, hp] = g[j]
    Gp = np.zeros((512, GW), dtype=np.float64)
    for k in range(4):
        a = min(max(128 * k - 5, 0), 512 - GW)
        Gp[128 * k:128 * k + 128, :] = G[128 * k:128 * k + 128, a:a + GW]
    W1p = np.zeros((512, 128), dtype=np.float64)
    Wcp = np.zeros((512, 128), dtype=np.float64)
    for k in range(4):
        for j in range(64):
            r0 = 128 * k + 2 * j
            W1p[r0, j] = 1.0
            W1p[r0 + 1, j] = 1.0
            W1p[r0, 64 + j] = 1.0
            W1p[r0 + 1, 64 + j] = -1.0
            Wcp[r0, j] = 0.5
            Wcp[r0 + 1, j] = 0.5
            Wcp[r0, 64 + j] = 0.5
            Wcp[r0 + 1, 64 + j] = -0.5
    bf = _np_bf16()
    Gf = G[0:128, :].copy()
    return Gp.astype(bf), W1p.astype(bf), Wcp.astype(bf), Gf.astype(bf)


def _conv_out_off(k):
    return min(max(128 * k - 5, 0), 512 - GW)


def _register_consts(nc, values, dtype=F32):
    for v in values:
        v = float(v)
        if (dtype, v) in nc.const_aps.aps:
            continue
        t = nc.alloc_sbuf_tensor(f"const-{dtype.name}-{v}", [128, 1], dtype)
        nc.gpsimd.memset(t.ap(), v)
        nc.const_aps.aps[(dtype, v)] = t.ap()
    nc.all_engine_barrier()


def build_nc():
    nc = bacc.Bacc()
    _register_consts(nc, [-T_LVL[1], -T_LVL[2], -T_LVL[3]])

    pred_d = nc.dram_tensor("pred", [NIMG, H, W], F32, kind="ExternalInput")
    targ_d = nc.dram_tensor("target", [NIMG, H, W], F32, kind="ExternalInput")
    gp_d = nc.dram_tensor("gp", [512, GW], BF16, kind="ExternalInput")
    gp2_d = nc.dram_tensor("gp2", [512, GW], BF16, kind="ExternalInput")
    gf_d = nc.dram_tensor("gf", [P, W], BF16, kind="ExternalInput")
    gf2_d = nc.dram_tensor("gf2", [P, W], BF16, kind="ExternalInput")
    w1p_d = nc.dram_tensor("w1p", [512, 128], BF16, kind="ExternalInput")
    wcp_d = nc.dram_tensor("wcp", [512, 128], BF16, kind="ExternalInput")
    out_d = nc.dram_tensor("out", [1, NACC], F32, kind="ExternalOutput")

    with TileContext(nc) as tc:
        with (
            tc.tile_pool(name="const", bufs=1) as cpool,
            tc.tile_pool(name="img", bufs=2) as ipool,
            tc.tile_pool(name="mid", bufs=2) as mpool,
            tc.tile_pool(name="tmp", bufs=3) as tpool,
            tc.tile_pool(name="wav", bufs=2) as wpool,
            tc.tile_pool(name="psum", bufs=1, space="PSUM") as pspool,
        ):
            # ---- constants ----
            gp = cpool.tile([P, 4, GW], BF16, tag="gp")
            gp2 = cpool.tile([P, 4, GW], BF16, tag="gp2")
            w1p = cpool.tile([P, 4, 128], BF16, tag="w1p")
            wcp = cpool.tile([P, 4, 128], BF16, tag="wcp")
            nc.sync.dma_start(gp[:], gp_d.rearrange("(c p) n -> p c n", p=P))
            nc.sync.dma_start(gp2[:], gp2_d.rearrange("(c p) n -> p c n", p=P))
            gf = cpool.tile([P, 1, W], BF16, tag="gf")
            gf2 = cpool.tile([P, 1, W], BF16, tag="gf2")
            nc.sync.dma_start(gf[:, 0, :], gf_d[:])
            nc.sync.dma_start(gf2[:, 0, :], gf2_d[:])
            nc.sync.dma_start(w1p[:], w1p_d.rearrange("(c p) n -> p c n", p=P))
            nc.sync.dma_start(wcp[:], wcp_d.rearrange("(c p) n -> p c n", p=P))

            gpc = cpool.tile([P, 4, GW], BF16, tag="gpc")
            gp2c = cpool.tile([P, 4, GW], BF16, tag="gp2c")
            w1pc = cpool.tile([P, 4, 128], BF16, tag="w1pc")
            wcpc = cpool.tile([P, 4, 128], BF16, tag="wcpc")
            nc.vector.tensor_copy(gpc[:], gp[:])
            nc.vector.tensor_copy(gp2c[:], gp2[:])
            gfc = cpool.tile([P, 1, W], BF16, tag="gfc")
            gf2c = cpool.tile([P, 1, W], BF16, tag="gf2c")
            nc.vector.tensor_copy(gfc[:], gf[:])
            nc.vector.tensor_copy(gf2c[:], gf2[:])
            nc.vector.tensor_copy(w1pc[:], w1p[:])
            nc.vector.tensor_copy(wcpc[:], wcp[:])
            gp, gp2, w1p, wcp = gpc, gp2c, w1pc, wcpc
            gf, gf2 = gfc, gf2c

            acc = cpool.tile([P, NACC], F32, tag="acc")
            nc.vector.memset(acc[:], 0.0)
            ones = cpool.tile([P, 1], F32, tag="ones")
            nc.vector.memset(ones[:], 1.0)
            zcol = cpool.tile([1, P], BF16, tag="zcol")
            nc.vector.memset(zcol[:], 0.0)
            zrow = cpool.tile([1, W], BF16, tag="zrow")
            nc.vector.memset(zrow[:], 0.0)

            for i in range(NIMG):
                # ---- stage-in ----
                p_t = ipool.tile([P, 4, W], BF16, tag="p")
                t_t = ipool.tile([P, 4, W], BF16, tag="t")
                nc.gpsimd.dma_start(
                    p_t[:], pred_d[i].rearrange("(c p) w -> p c w", p=P))
                nc.gpsimd.dma_start(
                    t_t[:], targ_d[i].rearrange("(c p) w -> p c w", p=P))

                p2_t = ipool.tile([P, 4, W], BF16, tag="p2")
                t2_t = ipool.tile([P, 4, W], BF16, tag="t2")
                pt2_t = ipool.tile([P, 4, W], BF16, tag="pt2")
                q_t = ipool.tile([P, 4, W], BF16, tag="q")
                nc.gpsimd.tensor_mul(p2_t[:], p_t[:], p_t[:])
                nc.gpsimd.tensor_mul(t2_t[:], t_t[:], t_t[:])
                # p*t (x2 folded into gp2 conv weights); tiny pre-touches
                # absorb the two DMA waits one at a time on gpsimd
                preg = tpool.tile([P, 2], BF16, tag="preg")
                nc.gpsimd.tensor_copy(preg[:, 0:1], p_t[:, 0, 0:1])
                nc.gpsimd.tensor_copy(preg[:, 1:2], t_t[:, 0, 0:1])
                nc.gpsimd.tensor_mul(pt2_t[:], p_t[:], t_t[:])
                # |p - t| -> L1 partial (sub on Pool; reduce on DVE)
                nc.gpsimd.tensor_sub(q_t[:], p_t[:], t_t[:])
                nc.vector.tensor_reduce(
                    acc[:, COL_L1 + i:COL_L1 + i + 1], q_t[:],
                    axis=mybir.AxisListType.XY, op=ALU.add,
                    apply_absolute_value=True)

                # ---- pass A: row conv + row pairs (data-form matmuls) ----
                rp = mpool.tile([P, 4, W], BF16, tag="rp")
                rt = mpool.tile([P, 4, W], BF16, tag="rt")
                rS = mpool.tile([P, 4, W], BF16, tag="rS")
                rD = mpool.tile([P, 4, W], BF16, tag="rD")
                rwp = mpool.tile([P, 4, W], BF16, tag="rwp")
                rwt = mpool.tile([P, 4, W], BF16, tag="rwt")
                for m in range(4):
                    bP = pspool.tile([P, W], F32, tag="ps0")
                    bT = pspool.tile([P, W], F32, tag="ps1")
                    bS = pspool.tile([P, W], F32, tag="ps2")
                    bD = pspool.tile([P, W], F32, tag="ps3")
                    bWp = pspool.tile([P, W], F32, tag="ps4")
                    bWt = pspool.tile([P, W], F32, tag="ps5")
                    nc.tensor.matmul(bWp[:], zcol[:], zrow[:],
                                     start=True, stop=False)
                    nc.tensor.matmul(bWt[:], zcol[:], zrow[:],
                                     start=True, stop=False)
                    for k in range(4):
                        a = _conv_out_off(k)
                        st = k == 0
                        mm = nc.tensor.matmul
                        pb = p_t[:, k, 128 * m:128 * m + 128]
                        tb = t_t[:, k, 128 * m:128 * m + 128]
                        if k == 0:
                            gw, g2w, sl = gf[:, 0, :], gf2[:, 0, :], slice(0, W)
                        else:
                            gw, g2w, sl = gp[:, k, :], gp2[:, k, :], \
                                slice(a, a + GW)
                        mm(bP[:, sl], pb, gw, start=st, stop=k == 3)
                        mm(bT[:, sl], tb, gw, start=st, stop=k == 3)
                        mm(bS[:, sl], p2_t[:, k, 128 * m:128 * m + 128],
                           gw, start=st, stop=False)
                        mm(bS[:, a:a + GW], t2_t[:, k, 128 * m:128 * m + 128],
                           gp[:, k, :], start=False, stop=k == 3)
                        mm(bD[:, sl], pt2_t[:, k, 128 * m:128 * m + 128],
                           g2w, start=st, stop=k == 3)
                        # Haar row pairs: RS cols [64k,64k+64), RD [256+64k,..)
                        mm(bWp[:, 64 * k:64 * k + 64], pb, w1p[:, k, 0:64],
                           start=False, stop=False)
                        mm(bWp[:, 256 + 64 * k:256 + 64 * k + 64], pb,
                           w1p[:, k, 64:128], start=False, stop=k == 3)
                        mm(bWt[:, 64 * k:64 * k + 64], tb, w1p[:, k, 0:64],
                           start=False, stop=False)
                        mm(bWt[:, 256 + 64 * k:256 + 64 * k + 64], tb,
                           w1p[:, k, 64:128], start=False, stop=k == 3)
                    nc.scalar.copy(rp[:, m, :], bP[:])
                    nc.scalar.copy(rt[:, m, :], bT[:])
                    nc.scalar.copy(rS[:, m, :], bS[:])
                    nc.scalar.copy(rD[:, m, :], bD[:])
                    nc.scalar.copy(rwp[:, m, :], bWp[:])
                    nc.vector.tensor_copy(rwt[:, m, :], bWt[:])

                # ---- pass B: col conv + col pairs; fused SSIM / wavelet ----
                cAp = wpool.tile([P, 2, 256], BF16, tag="cAp")
                cAt = wpool.tile([P, 2, 256], BF16, tag="cAt")
                for m in range(4):
                    bM1 = pspool.tile([P, W], F32, tag="ps0")
                    bM2 = pspool.tile([P, W], F32, tag="ps1")
                    bS2 = pspool.tile([P, W], F32, tag="ps2")
                    bD2 = pspool.tile([P, W], F32, tag="ps3")
                    bQp = pspool.tile([P, W], F32, tag="ps4")
                    bQt = pspool.tile([P, W], F32, tag="ps5")
                    nc.tensor.matmul(bQp[:], zcol[:], zrow[:],
                                     start=True, stop=False)
                    nc.tensor.matmul(bQt[:], zcol[:], zrow[:],
                                     start=True, stop=False)
                    for k in range(4):
                        a = _conv_out_off(k)
                        st = k == 0
                        mm = nc.tensor.matmul
                        if k == 0:
                            gw, sl = gf[:, 0, :], slice(0, W)
                        else:
                            gw, sl = gp[:, k, :], slice(a, a + GW)
                        mm(bM1[:, sl], rp[:, k, 128 * m:128 * m + 128],
                           gw, start=st, stop=k == 3)
                        mm(bM2[:, sl], rt[:, k, 128 * m:128 * m + 128],
                           gw, start=st, stop=k == 3)
                        mm(bS2[:, sl], rS[:, k, 128 * m:128 * m + 128],
                           gw, start=st, stop=k == 3)
                        mm(bD2[:, sl], rD[:, k, 128 * m:128 * m + 128],
                           gw, start=st, stop=k == 3)
                        mm(bQp[:, 64 * k:64 * k + 64],
                           rwp[:, k, 128 * m:128 * m + 128],
                           wcp[:, k, 0:64], start=False, stop=False)
                        mm(bQp[:, 256 + 64 * k:256 + 64 * k + 64],
                           rwp[:, k, 128 * m:128 * m + 128],
                           wcp[:, k, 64:128], start=False, stop=k == 3)
                        mm(bQt[:, 64 * k:64 * k + 64],
                           rwt[:, k, 128 * m:128 * m + 128],
                           wcp[:, k, 0:64], start=False, stop=False)
                        mm(bQt[:, 256 + 64 * k:256 + 64 * k + 64],
                           rwt[:, k, 128 * m:128 * m + 128],
                           wcp[:, k, 64:128], start=False, stop=k == 3)

                    # SSIM chain on this [128, 512] chunk
                    m1s = tpool.tile([P, W], BF16, tag="m1s")
                    sq1 = tpool.tile([P, W], BF16, tag="sq1")
                    sq2 = tpool.tile([P, W], BF16, tag="sq2")
                    n1p = tpool.tile([P, W], BF16, tag="n1p")
                    d1 = tpool.tile([P, W], BF16, tag="d1")
                    n2 = tpool.tile([P, W], BF16, tag="n2")
                    d2 = tpool.tile([P, W], BF16, tag="d2")
                    num = tpool.tile([P, W], BF16, tag="num")
                    den = tpool.tile([P, W], F32, tag="den")
                    sst = tpool.tile([P, W], BF16, tag="sst")
                    nc.vector.tensor_copy(m1s[:], bM1[:])
                    nc.vector.tensor_mul(sq1[:], m1s[:], m1s[:])
                    nc.scalar.activation(sq2[:], bM2[:], ACTF.Square)
                    stt = nc.vector.scalar_tensor_tensor
                    # n1p = 2*mu1*mu2
                    stt(n1p[:], bM2[:], 2.0, m1s[:], ALU.mult, ALU.mult)
                    # d1 = sq1 + sq2 (C1 folded into den/d2 forms)
                    nc.vector.tensor_add(d1[:], sq1[:], sq2[:])
                    # n2 = (D2 + C2) - n1p
                    stt(n2[:], bD2[:], C2, n1p[:], ALU.add, ALU.subtract)
                    # d2 = (S2 + C2) - d1   [= sigma1^2+sigma2^2+C2]
                    stt(d2[:], bS2[:], C2, d1[:], ALU.add, ALU.subtract)
                    # n1 = n1p + C1 ; num = n1 * n2
                    n1 = tpool.tile([P, W], BF16, tag="n1")
                    nc.vector.tensor_scalar_add(n1[:], n1p[:], C1)
                    nc.vector.tensor_mul(num[:], n1[:], n2[:])
                    # den = (d1 + C1) * d2 (f32); ssim = num * (1/den),
                    # summed via accum_out
                    stt(den[:], d1[:], C1, d2[:], ALU.add, ALU.mult)
                    rcp = tpool.tile([P, W], F32, tag="rcp")
                    nc.vector.reciprocal_approx_fast(rcp[:], den[:])
                    stt(sst[:], num[:], 0.0, rcp[:], ALU.bypass, ALU.mult,
                        accum_out=acc[:, COL_SSIM + 4 * i + m:
                                      COL_SSIM + 4 * i + m + 1])

                    # wavelet L1 quadrants of bQp/bQt
                    _wav_detail(nc, tpool, acc, COL_W1 + 4 * i + m,
                                bQp, bQt, m, cAp, cAt, T_LVL[1])

                # ---- wavelet level 2 on cA [256,256] ----
                cA2p, cA2t = _wav_level2(nc, tc, wpool, tpool, pspool,
                                         w1p, wcp, acc, i, cAp, cAt,
                                         zcol, zrow)
                # ---- wavelet level 3 on cA2 [128,128] ----
                _wav_level3(nc, wpool, tpool, pspool, w1p, wcp, acc, i,
                            cA2p, cA2t, zcol, zrow)

            # ---- final reduction: out = ones^T @ acc ----
            outp = pspool.tile([1, NACC], F32, tag="outp")
            nc.tensor.matmul(outp[:], ones[:], acc[:], start=True, stop=True)
            outs = cpool.tile([1, NACC], F32, tag="outs")
            nc.scalar.copy(outs[:], outp[:])
            nc.sync.dma_start(out_d[:], outs[:])

    nc.finalize()
    return nc


def _soft_diff_sum(nc, tpool, acc_col_ap, fp, ft, thr, tag):
    """acc_col += sum |soft(fp) - soft(ft)| over a detail field.

    fp/ft are PSUM (or SBUF) APs of identical shape [pp, n].
    soft(x) = relu(x - T) - relu(-x - T).
    """
    pp = fp.shape[0]
    n = int(np.prod(fp.shape[1:]))
    spp = tpool.tile([pp, n], BF16, tag="spp")
    spn = tpool.tile([pp, n], BF16, tag="spn")
    stp = tpool.tile([pp, n], BF16, tag="stp")
    stn = tpool.tile([pp, n], BF16, tag="stn")
    q1 = tpool.tile([pp, n], BF16, tag="wq1")
    q2 = tpool.tile([pp, n], BF16, tag="wq2")
    q3 = tpool.tile([pp, n], BF16, tag="wq3")
    act = nc.scalar.activation
    act(spp[:], fp, ACTF.Relu, bias=-thr, scale=1.0)
    act(spn[:], fp, ACTF.Relu, bias=-thr, scale=-1.0)
    act(stp[:], ft, ACTF.Relu, bias=-thr, scale=1.0)
    act(stn[:], ft, ACTF.Relu, bias=-thr, scale=-1.0)
    nc.vector.scalar_tensor_tensor(
        q1[:], spp[:], 0.0, stp[:], ALU.bypass, ALU.subtract)
    nc.gpsimd.tensor_sub(q2[:], spn[:], stn[:])
    nc.vector.scalar_tensor_tensor(
        q3[:], q1[:], 0.0, q2[:], ALU.bypass, ALU.subtract)
    nc.vector.tensor_reduce(
        acc_col_ap, q3[:], axis=mybir.AxisListType.X, op=ALU.add,
        apply_absolute_value=True)


def _wav_detail(nc, tpool, acc, col, bQp, bQt, m, cAp, cAt, thr):
    """Handle one [128,512] chunk of the level-1 DWT output.

    m in {0,1}: rows are RS -> cols [0,256)=cA (save), [256,512)=cV (detail).
    m in {2,3}: rows are RD -> cH | cD, both detail.
    """
    if m < 2:
        nc.scalar.copy(cAp[:, m, :], bQp[:, 0:256])
        nc.scalar.copy(cAt[:, m, :], bQt[:, 0:256])
        _soft_diff_sum(nc, tpool, acc[:, col:col + 1],
                       bQp[:, 256:512], bQt[:, 256:512], thr, "a")
    else:
        _soft_diff_sum(nc, tpool, acc[:, col:col + 1],
                       bQp[:], bQt[:], thr, "b")


def _wav_level2(nc, tc, wpool, tpool, pspool, w1p, wcp, acc, i, cAp, cAt,
                zcol, zrow):
    """Level-2 DWT on cA [256,256] (stored [128, 2, 256])."""
    rw2p = wpool.tile([P, 2, 256], BF16, tag="rw2p")
    rw2t = wpool.tile([P, 2, 256], BF16, tag="rw2t")
    for m in range(2):
        b2p = pspool.tile([P, 256], F32, tag="ps0")
        b2t = pspool.tile([P, 256], F32, tag="ps1")
        nc.tensor.matmul(b2p[:], zcol[:], zrow[:, 0:256], start=True, stop=False)
        nc.tensor.matmul(b2t[:], zcol[:], zrow[:, 0:256], start=True, stop=False)
        for k in range(2):
            st = False
            mm = nc.tensor.matmul
            mm(b2p[:, 64 * k:64 * k + 64],
               cAp[:, k, 128 * m:128 * m + 128], w1p[:, k, 0:64],
               start=st, stop=False)
            mm(b2p[:, 128 + 64 * k:128 + 64 * k + 64],
               cAp[:, k, 128 * m:128 * m + 128], w1p[:, k, 64:128],
               start=False, stop=k == 1)
            mm(b2t[:, 64 * k:64 * k + 64],
               cAt[:, k, 128 * m:128 * m + 128], w1p[:, k, 0:64],
               start=st, stop=False)
            mm(b2t[:, 128 + 64 * k:128 + 64 * k + 64],
               cAt[:, k, 128 * m:128 * m + 128], w1p[:, k, 64:128],
               start=False, stop=k == 1)
        nc.scalar.copy(rw2p[:, m, :], b2p[:])
        nc.vector.tensor_copy(rw2t[:, m, :], b2t[:])

    cA2p = wpool.tile([P, 128], BF16, tag="cA2p")
    cA2t = wpool.tile([P, 128], BF16, tag="cA2t")
    for m in range(2):
        d2p = pspool.tile([P, 256], F32, tag="ps2")
        d2t = pspool.tile([P, 256], F32, tag="ps3")
        nc.tensor.matmul(d2p[:], zcol[:], zrow[:, 0:256], start=True, stop=False)
        nc.tensor.matmul(d2t[:], zcol[:], zrow[:, 0:256], start=True, stop=False)
        for k in range(2):
            st = False
            mm = nc.tensor.matmul
            mm(d2p[:, 64 * k:64 * k + 64],
               rw2p[:, k, 128 * m:128 * m + 128], wcp[:, k, 0:64],
               start=st, stop=False)
            mm(d2p[:, 128 + 64 * k:128 + 64 * k + 64],
               rw2p[:, k, 128 * m:128 * m + 128], wcp[:, k, 64:128],
               start=False, stop=k == 1)
            mm(d2t[:, 64 * k:64 * k + 64],
               rw2t[:, k, 128 * m:128 * m + 128], wcp[:, k, 0:64],
               start=st, stop=False)
            mm(d2t[:, 128 + 64 * k:128 + 64 * k + 64],
               rw2t[:, k, 128 * m:128 * m + 128], wcp[:, k, 64:128],
               start=False, stop=k == 1)
        col = COL_W2 + 2 * i + m
        if m == 0:
            nc.scalar.copy(cA2p[:], d2p[:, 0:128])
            nc.scalar.copy(cA2t[:], d2t[:, 0:128])
            _soft_diff_sum(nc, tpool, acc[:, col:col + 1],
                           d2p[:, 128:256], d2t[:, 128:256], T_LVL[2], "c")
        else:
            _soft_diff_sum(nc, tpool, acc[:, col:col + 1],
                           d2p[:], d2t[:], T_LVL[2], "d")
    return cA2p, cA2t


def _wav_level3(nc, wpool, tpool, pspool, w1p, wcp, acc, i, cA2p, cA2t,
                zcol, zrow):
    """Level-3 DWT on cA2 [128,128]."""
    rw3p = wpool.tile([P, 128], BF16, tag="rw3p")
    rw3t = wpool.tile([P, 128], BF16, tag="rw3t")
    b3p = pspool.tile([P, 128], F32, tag="ps0")
    b3t = pspool.tile([P, 128], F32, tag="ps1")
    mm = nc.tensor.matmul
    mm(b3p[:], zcol[:], zrow[:, 0:128], start=True, stop=False)
    mm(b3t[:], zcol[:], zrow[:, 0:128], start=True, stop=False)
    mm(b3p[:, 0:64], cA2p[:], w1p[:, 0, 0:64], start=False, stop=False)
    mm(b3p[:, 64:128], cA2p[:], w1p[:, 0, 64:128], start=False, stop=True)
    mm(b3t[:, 0:64], cA2t[:], w1p[:, 0, 0:64], start=False, stop=False)
    mm(b3t[:, 64:128], cA2t[:], w1p[:, 0, 64:128], start=False, stop=True)
    nc.scalar.copy(rw3p[:], b3p[:])
    nc.vector.tensor_copy(rw3t[:], b3t[:])

    d3p = pspool.tile([P, 128], F32, tag="ps2")
    d3t = pspool.tile([P, 128], F32, tag="ps3")
    mm(d3p[:], zcol[:], zrow[:, 0:128], start=True, stop=False)
    mm(d3t[:], zcol[:], zrow[:, 0:128], start=True, stop=False)
    mm(d3p[:, 0:64], rw3p[:], wcp[:, 0, 0:64], start=False, stop=False)
    mm(d3p[:, 64:128], rw3p[:], wcp[:, 0, 64:128], start=False, stop=True)
    mm(d3t[:, 0:64], rw3t[:], wcp[:, 0, 0:64], start=False, stop=False)
    mm(d3t[:, 64:128], rw3t[:], wcp[:, 0, 64:128], start=False, stop=True)
    # quadrants: partitions 0:64 = RS rows (cA3 | cV3), 64:128 = RD (cH3|cD3)
    # detail fields: cV3 = [0:64, 64:128], cH3+cD3 = [64:128, 0:128]
    col = COL_W3 + 2 * i
    _soft_diff_sum(nc, tpool, acc[0:64, col:col + 1],
                   d3p[0:64, 64:128], d3t[0:64, 64:128], T_LVL[3], "e")
    _soft_diff_sum(nc, tpool, acc[64:128, col + 1:col + 2],
                   d3p[64:128, 0:128], d3t[64:128, 0:128], T_LVL[3], "f")


def make_in_maps(pred, target):
    """pred/target: [32, 512, 512] f32 -> list of 8 per-core input dicts."""
    gp, w1p, wcp, gf = _build_consts()
    gp2 = (gp.astype(np.float32) * 2.0).astype(_np_bf16())
    gf2 = (gf.astype(np.float32) * 2.0).astype(_np_bf16())
    maps = []
    for c in range(NCORES):
        maps.append({
            "pred": np.ascontiguousarray(pred[NIMG * c:NIMG * (c + 1)]),
            "target": np.ascontiguousarray(target[NIMG * c:NIMG * (c + 1)]),
            "gp": gp, "gp2": gp2, "gf": gf, "gf2": gf2,
            "w1p": w1p, "wcp": wcp,
        })
    return maps


_NC_CACHE = None


def _get_nc():
    global _NC_CACHE
    if _NC_CACHE is None:
        _NC_CACHE = build_nc()
    return _NC_CACHE


def kernel(pred: np.ndarray, target: np.ndarray) -> np.ndarray:
    from concourse.bass_utils import run_bass_kernel_spmd

    pred = np.ascontiguousarray(np.asarray(pred, dtype=np.float32)
                                .reshape(32, H, W))
    target = np.ascontiguousarray(np.asarray(target, dtype=np.float32)
                                  .reshape(32, H, W))
    in_maps = make_in_maps(pred, target)

    nc = _get_nc()
    res = run_bass_kernel_spmd(nc, in_maps, core_ids=list(range(NCORES)))
    partials = np.stack([r["out"][0].astype(np.float64)
                         for r in res.results])  # [8, 64]
    tot = partials.sum(axis=0)

    npix = 32.0 * H * W
    l1 = tot[COL_L1:COL_L1 + 4].sum() / npix
    ssim_mean = tot[COL_SSIM:COL_SSIM + 16].sum() / npix
    ssim_loss = np.clip(1.0 - ssim_mean, 0.0, 2.0)
    w1 = tot[COL_W1:COL_W1 + 16].sum()   # finest: 256^2 bands
    w2 = tot[COL_W2:COL_W2 + 8].sum()    # 128^2 bands
    w3 = tot[COL_W3:COL_W3 + 8].sum()    # coarsest: 64^2 bands
    wav = (
        (w3 / (32.0 * 64 * 64) / 3.0) / 1.0
        + (w2 / (32.0 * 128 * 128) / 3.0) / 2.0
        + (w1 / (32.0 * 256 * 256) / 3.0) / 3.0
    )
    loss = l1 + 0.5 * ssim_loss + 0.1 * wav
    return np.float32(loss)


# revision 34
# speedup vs baseline: 49.4873x; 1.1703x over previous
"""Trainium2 Bass kernel for nn_CombinedLoss (L1 + 0.5*SSIM + 0.1*Wavelet).

Sharding: pure data-parallel over batch (32 images -> 4 per core x 8 cores).
Each core returns a [1, 64] f32 vector of partial sums; host combines.

On-chip plan per core (4 images, 512x512, bf16 data / f32 PSUM):
  - stage-in: DMA-cast f32->bf16; p^2/t^2/pt and p-t on GPSIMD (Pool);
    L1 |p-t| via DVE reduce(abs)
  - pass A (PE): row-direction conv for {p, t, p^2+t^2, 2pt} + Haar row-pairs
    for {p, t}, via "data-form" matmuls (lhsT = image blocks, rhs = packed
    banded Gaussian Gp / pair matrix W1p). Output comes out transposed.
  - pass B (PE): column-direction conv / Haar col-pairs on the transposed
    intermediates -> full conv fields mu1, mu2, S2=conv(p^2+t^2), D2=2conv(pt)
    and DWT level-1 quadrants, directly in natural orientation.
  - SSIM map: DVE scalar_tensor_tensor/TT chain with folded constants,
    reciprocal_approx_fast for the division, accum_out for the sum.
  - Wavelet levels 2,3: same two-pass machinery on the cA quadrant.
    Soft-threshold via ACT relus, diffs via STT, sum via reduce(abs).
"""

import sys

sys.path.insert(0, "/opt/trn_rl_repo")

import numpy as np

import concourse.bass as bass
import concourse.bacc as bacc
import concourse.mybir as mybir
from concourse.tile import TileContext

F32 = mybir.dt.float32
BF16 = mybir.dt.bfloat16
ALU = mybir.AluOpType
ACTF = mybir.ActivationFunctionType

P = 128
H = W = 512
NIMG = 4          # images per core
NCORES = 8
WIN = 11
SIGMA = 1.5
C1 = 0.01 ** 2
C2 = 0.03 ** 2
C12 = C1 + C2
GW = 138          # padded conv band window width (128 + 2*5)

# wavelet thresholds: my level L (1=finest 256^2 bands) maps to reference
# level_idx (1=coarsest): ref_idx = 4 - L
T_LVL = {1: (50.0 / 4.0) / 255.0, 2: (50.0 / 2.0) / 255.0, 3: 50.0 / 255.0}

# accumulator column map (acc is [128, 64] f32; out = ones^T @ acc -> [1,64])
COL_L1 = 0        # + img               (4)
COL_SSIM = 4      # + 4*img + m         (16)
COL_W1 = 20       # + 4*img + m         (16)
COL_W2 = 36       # + 2*img + m2        (8)
COL_W3 = 44       # + 2*img + {0,1}     (8)
NACC = 64


def _np_bf16():
    return mybir.dt.np(BF16)


def _gauss_taps():
    """11 Gaussian taps, bf16-quantized with the quantization residual
    redistributed so the bf16 tap-sum matches the f32 tap-sum (a tap-sum
    error gamma biases sigma12 by -2*gamma*mu1*mu2, which is large relative
    to the tiny ssim_map mean)."""
    x = np.arange(WIN, dtype=np.float32) - WIN // 2
    g32 = np.exp(-(x ** 2) / (2.0 * np.float32(SIGMA) ** 2))
    g32 = g32 / g32.sum()
    bf = _np_bf16()
    gb = g32.astype(bf)
    target = g32.astype(np.float64).sum()
    for _ in range(40):
        gamma = gb.astype(np.float64).sum() - target
        if abs(gamma) < 1e-7:
            break
        best = None
        for i in range(WIN):
            v = gb[i]
            hi = np.asarray(10.0, dtype=bf)
            lo = np.asarray(-10.0, dtype=bf)
            for cand in (np.nextafter(v, hi, dtype=bf),
                         np.nextafter(v, lo, dtype=bf)):
                g2 = gb.copy()
                g2[i] = cand
                newg = abs(g2.astype(np.float64).sum() - target)
                drift = abs(float(cand) - g32[i]) / g32[i]
                if newg < abs(gamma) and drift < 0.01 and (
                        best is None or newg < best[0]):
                    best = (newg, i, cand)
        if best is None:
            break
        gb[best[1]] = best[2]
    return gb.astype(np.float64)


def _build_consts():
    """Packed conv band Gp [512,138], Haar row W1p [512,128] (+-1),
    Haar col Wcp [512,128] (+-0.5)."""
    g = _gauss_taps()
    G = np.zeros((512, 512), dtype=np.float64)
    for h in range(512):
        for j in range(WIN):
            hp = h + j - WIN // 2
            if 0 <= hp < 512:
                G[h, hp] = g[j]
    Gp = np.zeros((512, GW), dtype=np.float64)
    for k in range(4):
        a = min(max(128 * k - 5, 0), 512 - GW)
        Gp[128 * k:128 * k + 128, :] = G[128 * k:128 * k + 128, a:a + GW]
    W1p = np.zeros((512, 128), dtype=np.float64)
    Wcp = np.zeros((512, 128), dtype=np.float64)
    for k in range(4):
        for j in range(64):
            r0 = 128 * k + 2 * j
            W1p[r0, j] = 1.0
            W1p[r0 + 1, j] = 1.0
            W1p[r0, 64 + j] = 1.0
            W1p[r0 + 1, 64 + j] = -1.0
            Wcp[r0, j] = 0.5
            Wcp[r0 + 1, j] = 0.5
            Wcp[r0, 64 + j] = 0.5
            Wcp[r0 + 1, 64 + j] = -0.5
    bf = _np_bf16()
    Gf = G[0:128, :].copy()
    return Gp.astype(bf), W1p.astype(bf), Wcp.astype(bf), Gf.astype(bf)


def _conv_out_off(k):
    return min(max(128 * k - 5, 0), 512 - GW)


def _register_consts(nc, values, dtype=F32):
    for v in values:
        v = float(v)
        if (dtype, v) in nc.const_aps.aps:
            continue
        t = nc.alloc_sbuf_tensor(f"const-{dtype.name}-{v}", [128, 1], dtype)
        nc.gpsimd.memset(t.ap(), v)
        nc.const_aps.aps[(dtype, v)] = t.ap()
    nc.all_engine_barrier()


def build_nc():
    nc = bacc.Bacc()
    _register_consts(nc, [-T_LVL[1], -T_LVL[2], -T_LVL[3]])

    pred_d = nc.dram_tensor("pred", [NIMG, H, W], F32, kind="ExternalInput")
    targ_d = nc.dram_tensor("target", [NIMG, H, W], F32, kind="ExternalInput")
    gp_d = nc.dram_tensor("gp", [512, GW], BF16, kind="ExternalInput")
    gp2_d = nc.dram_tensor("gp2", [512, GW], BF16, kind="ExternalInput")
    gf_d = nc.dram_tensor("gf", [P, W], BF16, kind="ExternalInput")
    gf2_d = nc.dram_tensor("gf2", [P, W], BF16, kind="ExternalInput")
    w1p_d = nc.dram_tensor("w1p", [512, 128], BF16, kind="ExternalInput")
    wcp_d = nc.dram_tensor("wcp", [512, 128], BF16, kind="ExternalInput")
    out_d = nc.dram_tensor("out", [1, NACC], F32, kind="ExternalOutput")

    with TileContext(nc) as tc:
        with (
            tc.tile_pool(name="const", bufs=1) as cpool,
            tc.tile_pool(name="img", bufs=2) as ipool,
            tc.tile_pool(name="mid", bufs=2) as mpool,
            tc.tile_pool(name="tmp", bufs=4) as tpool,
            tc.tile_pool(name="wav", bufs=2) as wpool,
            tc.tile_pool(name="psum", bufs=1, space="PSUM") as pspool,
        ):
            # ---- constants ----
            gp = cpool.tile([P, 4, GW], BF16, tag="gp")
            gp2 = cpool.tile([P, 4, GW], BF16, tag="gp2")
            w1p = cpool.tile([P, 4, 128], BF16, tag="w1p")
            wcp = cpool.tile([P, 4, 128], BF16, tag="wcp")
            nc.sync.dma_start(gp[:], gp_d.rearrange("(c p) n -> p c n", p=P))
            nc.sync.dma_start(gp2[:], gp2_d.rearrange("(c p) n -> p c n", p=P))
            gf = cpool.tile([P, 1, W], BF16, tag="gf")
            gf2 = cpool.tile([P, 1, W], BF16, tag="gf2")
            nc.sync.dma_start(gf[:, 0, :], gf_d[:])
            nc.sync.dma_start(gf2[:, 0, :], gf2_d[:])
            nc.sync.dma_start(w1p[:], w1p_d.rearrange("(c p) n -> p c n", p=P))
            nc.sync.dma_start(wcp[:], wcp_d.rearrange("(c p) n -> p c n", p=P))

            gpc = cpool.tile([P, 4, GW], BF16, tag="gpc")
            gp2c = cpool.tile([P, 4, GW], BF16, tag="gp2c")
            w1pc = cpool.tile([P, 4, 128], BF16, tag="w1pc")
            wcpc = cpool.tile([P, 4, 128], BF16, tag="wcpc")
            nc.vector.tensor_copy(gpc[:], gp[:])
            nc.vector.tensor_copy(gp2c[:], gp2[:])
            gfc = cpool.tile([P, 1, W], BF16, tag="gfc")
            gf2c = cpool.tile([P, 1, W], BF16, tag="gf2c")
            nc.vector.tensor_copy(gfc[:], gf[:])
            nc.vector.tensor_copy(gf2c[:], gf2[:])
            nc.vector.tensor_copy(w1pc[:], w1p[:])
            nc.vector.tensor_copy(wcpc[:], wcp[:])
            gp, gp2, w1p, wcp = gpc, gp2c, w1pc, wcpc
            gf, gf2 = gfc, gf2c

            acc = cpool.tile([P, NACC], F32, tag="acc")
            nc.vector.memset(acc[:], 0.0)
            ones = cpool.tile([P, 1], F32, tag="ones")
            nc.vector.memset(ones[:], 1.0)
            zcol = cpool.tile([1, P], BF16, tag="zcol")
            nc.vector.memset(zcol[:], 0.0)
            zrow = cpool.tile([1, W], BF16, tag="zrow")
            nc.vector.memset(zrow[:], 0.0)

            for i in range(NIMG):
                # ---- stage-in ----
                p_t = ipool.tile([P, 4, W], BF16, tag="p")
                t_t = ipool.tile([P, 4, W], BF16, tag="t")
                nc.gpsimd.dma_start(
                    p_t[:], pred_d[i].rearrange("(c p) w -> p c w", p=P))
                nc.gpsimd.dma_start(
                    t_t[:], targ_d[i].rearrange("(c p) w -> p c w", p=P))

                p2_t = ipool.tile([P, 4, W], BF16, tag="p2")
                t2_t = ipool.tile([P, 4, W], BF16, tag="t2")
                pt2_t = ipool.tile([P, 4, W], BF16, tag="pt2")
                q_t = ipool.tile([P, 4, W], BF16, tag="q")
                nc.gpsimd.tensor_mul(p2_t[:], p_t[:], p_t[:])
                nc.gpsimd.tensor_mul(t2_t[:], t_t[:], t_t[:])
                # p*t (x2 folded into gp2 conv weights); tiny pre-touches
                # absorb the two DMA waits one at a time on gpsimd
                preg = tpool.tile([P, 2], BF16, tag="preg")
                nc.gpsimd.tensor_copy(preg[:, 0:1], p_t[:, 0, 0:1])
                nc.gpsimd.tensor_copy(preg[:, 1:2], t_t[:, 0, 0:1])
                nc.gpsimd.tensor_mul(pt2_t[:], p_t[:], t_t[:])
                # |p - t| -> L1 partial (sub on Pool; reduce on DVE)
                nc.gpsimd.tensor_sub(q_t[:], p_t[:], t_t[:])
                nc.vector.tensor_reduce(
                    acc[:, COL_L1 + i:COL_L1 + i + 1], q_t[:],
                    axis=mybir.AxisListType.XY, op=ALU.add,
                    apply_absolute_value=True)

                # ---- pass A: row conv + row pairs (data-form matmuls) ----
                rp = mpool.tile([P, 4, W], BF16, tag="rp")
                rt = mpool.tile([P, 4, W], BF16, tag="rt")
                rS = mpool.tile([P, 4, W], BF16, tag="rS")
                rD = mpool.tile([P, 4, W], BF16, tag="rD")
                rwp = mpool.tile([P, 4, W], BF16, tag="rwp")
                rwt = mpool.tile([P, 4, W], BF16, tag="rwt")
                for m in range(4):
                    bP = pspool.tile([P, W], F32, tag="ps0")
                    bT = pspool.tile([P, W], F32, tag="ps1")
                    bS = pspool.tile([P, W], F32, tag="ps2")
                    bD = pspool.tile([P, W], F32, tag="ps3")
                    bWp = pspool.tile([P, W], F32, tag="ps4")
                    bWt = pspool.tile([P, W], F32, tag="ps5")
                    nc.tensor.matmul(bWp[:], zcol[:], zrow[:],
                                     start=True, stop=False)
                    nc.tensor.matmul(bWt[:], zcol[:], zrow[:],
                                     start=True, stop=False)
                    for k in range(4):
                        a = _conv_out_off(k)
                        st = k == 0
                        mm = nc.tensor.matmul
                        pb = p_t[:, k, 128 * m:128 * m + 128]
                        tb = t_t[:, k, 128 * m:128 * m + 128]
                        if k == 0:
                            gw, g2w, sl = gf[:, 0, :], gf2[:, 0, :], slice(0, W)
                        else:
                            gw, g2w, sl = gp[:, k, :], gp2[:, k, :], \
                                slice(a, a + GW)
                        mm(bP[:, sl], pb, gw, start=st, stop=k == 3)
                        mm(bT[:, sl], tb, gw, start=st, stop=k == 3)
                        mm(bS[:, sl], p2_t[:, k, 128 * m:128 * m + 128],
                           gw, start=st, stop=False)
                        mm(bS[:, a:a + GW], t2_t[:, k, 128 * m:128 * m + 128],
                           gp[:, k, :], start=False, stop=k == 3)
                        mm(bD[:, sl], pt2_t[:, k, 128 * m:128 * m + 128],
                           g2w, start=st, stop=k == 3)
                        # Haar row pairs: RS cols [64k,64k+64), RD [256+64k,..)
                        mm(bWp[:, 64 * k:64 * k + 64], pb, w1p[:, k, 0:64],
                           start=False, stop=False)
                        mm(bWp[:, 256 + 64 * k:256 + 64 * k + 64], pb,
                           w1p[:, k, 64:128], start=False, stop=k == 3)
                        mm(bWt[:, 64 * k:64 * k + 64], tb, w1p[:, k, 0:64],
                           start=False, stop=False)
                        mm(bWt[:, 256 + 64 * k:256 + 64 * k + 64], tb,
                           w1p[:, k, 64:128], start=False, stop=k == 3)
                    nc.scalar.copy(rp[:, m, :], bP[:])
                    nc.scalar.copy(rt[:, m, :], bT[:])
                    nc.scalar.copy(rS[:, m, :], bS[:])
                    nc.scalar.copy(rD[:, m, :], bD[:])
                    nc.scalar.copy(rwp[:, m, :], bWp[:])
                    nc.vector.tensor_copy(rwt[:, m, :], bWt[:])

                # ---- pass B: col conv + col pairs; fused SSIM / wavelet ----
                cAp = wpool.tile([P, 2, 256], BF16, tag="cAp")
                cAt = wpool.tile([P, 2, 256], BF16, tag="cAt")
                for m in range(4):
                    bM1 = pspool.tile([P, W], F32, tag="ps0")
                    bM2 = pspool.tile([P, W], F32, tag="ps1")
                    bS2 = pspool.tile([P, W], F32, tag="ps2")
                    bD2 = pspool.tile([P, W], F32, tag="ps3")
                    bQp = pspool.tile([P, W], F32, tag="ps4")
                    bQt = pspool.tile([P, W], F32, tag="ps5")
                    nc.tensor.matmul(bQp[:], zcol[:], zrow[:],
                                     start=True, stop=False)
                    nc.tensor.matmul(bQt[:], zcol[:], zrow[:],
                                     start=True, stop=False)
                    for k in range(4):
                        a = _conv_out_off(k)
                        st = k == 0
                        mm = nc.tensor.matmul
                        if k == 0:
                            gw, sl = gf[:, 0, :], slice(0, W)
                        else:
                            gw, sl = gp[:, k, :], slice(a, a + GW)
                        mm(bM1[:, sl], rp[:, k, 128 * m:128 * m + 128],
                           gw, start=st, stop=k == 3)
                        mm(bM2[:, sl], rt[:, k, 128 * m:128 * m + 128],
                           gw, start=st, stop=k == 3)
                        mm(bS2[:, sl], rS[:, k, 128 * m:128 * m + 128],
                           gw, start=st, stop=k == 3)
                        mm(bD2[:, sl], rD[:, k, 128 * m:128 * m + 128],
                           gw, start=st, stop=k == 3)
                        mm(bQp[:, 64 * k:64 * k + 64],
                           rwp[:, k, 128 * m:128 * m + 128],
                           wcp[:, k, 0:64], start=False, stop=False)
                        mm(bQp[:, 256 + 64 * k:256 + 64 * k + 64],
                           rwp[:, k, 128 * m:128 * m + 128],
                           wcp[:, k, 64:128], start=False, stop=k == 3)
                        mm(bQt[:, 64 * k:64 * k + 64],
                           rwt[:, k, 128 * m:128 * m + 128],
                           wcp[:, k, 0:64], start=False, stop=False)
                        mm(bQt[:, 256 + 64 * k:256 + 64 * k + 64],
                           rwt[:, k, 128 * m:128 * m + 128],
                           wcp[:, k, 64:128], start=False, stop=k == 3)

                    # SSIM chain on this [128, 512] chunk
                    m1s = tpool.tile([P, W], BF16, tag="m1s")
                    sq1 = tpool.tile([P, W], BF16, tag="sq1")
                    sq2 = tpool.tile([P, W], BF16, tag="sq2")
                    n1p = tpool.tile([P, W], BF16, tag="n1p")
                    d1 = tpool.tile([P, W], BF16, tag="d1")
                    n2 = tpool.tile([P, W], BF16, tag="n2")
                    d2 = tpool.tile([P, W], BF16, tag="d2")
                    num = tpool.tile([P, W], BF16, tag="num")
                    den = tpool.tile([P, W], F32, tag="den")
                    sst = tpool.tile([P, W], BF16, tag="sst")
                    nc.vector.tensor_copy(m1s[:], bM1[:])
                    nc.gpsimd.tensor_mul(sq1[:], m1s[:], m1s[:])
                    nc.scalar.activation(sq2[:], bM2[:], ACTF.Square)
                    stt = nc.vector.scalar_tensor_tensor
                    # n1p = 2*mu1*mu2
                    stt(n1p[:], bM2[:], 2.0, m1s[:], ALU.mult, ALU.mult)
                    # d1 = sq1 + sq2 (C1 folded into den/d2 forms)
                    nc.gpsimd.tensor_add(d1[:], sq1[:], sq2[:])
                    # n2 = (D2 + C2) - n1p
                    stt(n2[:], bD2[:], C2, n1p[:], ALU.add, ALU.subtract)
                    # d2 = (S2 + C2) - d1   [= sigma1^2+sigma2^2+C2]
                    stt(d2[:], bS2[:], C2, d1[:], ALU.add, ALU.subtract)
                    # n1 = n1p + C1 ; num = n1 * n2
                    n1 = tpool.tile([P, W], BF16, tag="n1")
                    nc.vector.tensor_scalar_add(n1[:], n1p[:], C1)
                    nc.gpsimd.tensor_mul(num[:], n1[:], n2[:])
                    # den = (d1 + C1) * d2 (f32); ssim = num * (1/den),
                    # summed via accum_out
                    stt(den[:], d1[:], C1, d2[:], ALU.add, ALU.mult)
                    rcp = tpool.tile([P, W], F32, tag="rcp")
                    nc.vector.reciprocal_approx_fast(rcp[:], den[:])
                    stt(sst[:], num[:], 0.0, rcp[:], ALU.bypass, ALU.mult,
                        accum_out=acc[:, COL_SSIM + 4 * i + m:
                                      COL_SSIM + 4 * i + m + 1])

                    # wavelet L1 quadrants of bQp/bQt
                    _wav_detail(nc, tpool, acc, COL_W1 + 4 * i + m,
                                bQp, bQt, m, cAp, cAt, T_LVL[1])

                # ---- wavelet level 2 on cA [256,256] ----
                cA2p, cA2t = _wav_level2(nc, tc, wpool, tpool, pspool,
                                         w1p, wcp, acc, i, cAp, cAt,
                                         zcol, zrow)
                # ---- wavelet level 3 on cA2 [128,128] ----
                _wav_level3(nc, wpool, tpool, pspool, w1p, wcp, acc, i,
                            cA2p, cA2t, zcol, zrow)

            # ---- final reduction: out = ones^T @ acc ----
            outp = pspool.tile([1, NACC], F32, tag="outp")
            nc.tensor.matmul(outp[:], ones[:], acc[:], start=True, stop=True)
            outs = cpool.tile([1, NACC], F32, tag="outs")
            nc.scalar.copy(outs[:], outp[:])
            nc.sync.dma_start(out_d[:], outs[:])

    nc.finalize()
    return nc


def _soft_diff_sum(nc, tpool, acc_col_ap, fp, ft, thr, tag):
    """acc_col += sum |soft(fp) - soft(ft)| over a detail field.

    fp/ft are PSUM (or SBUF) APs of identical shape [pp, n].
    soft(x) = relu(x - T) - relu(-x - T).
    """
    pp = fp.shape[0]
    n = int(np.prod(fp.shape[1:]))
    spp = tpool.tile([pp, n], BF16, tag="spp")
    spn = tpool.tile([pp, n], BF16, tag="spn")
    stp = tpool.tile([pp, n], BF16, tag="stp")
    stn = tpool.tile([pp, n], BF16, tag="stn")
    q1 = tpool.tile([pp, n], BF16, tag="wq1")
    q2 = tpool.tile([pp, n], BF16, tag="wq2")
    q3 = tpool.tile([pp, n], BF16, tag="wq3")
    act = nc.scalar.activation
    act(spp[:], fp, ACTF.Relu, bias=-thr, scale=1.0)
    act(spn[:], fp, ACTF.Relu, bias=-thr, scale=-1.0)
    act(stp[:], ft, ACTF.Relu, bias=-thr, scale=1.0)
    act(stn[:], ft, ACTF.Relu, bias=-thr, scale=-1.0)
    nc.gpsimd.tensor_sub(q1[:], spp[:], stp[:])
    nc.gpsimd.tensor_sub(q2[:], spn[:], stn[:])
    nc.gpsimd.tensor_sub(q3[:], q1[:], q2[:])
    nc.vector.tensor_reduce(
        acc_col_ap, q3[:], axis=mybir.AxisListType.X, op=ALU.add,
        apply_absolute_value=True)


def _wav_detail(nc, tpool, acc, col, bQp, bQt, m, cAp, cAt, thr):
    """Handle one [128,512] chunk of the level-1 DWT output.

    m in {0,1}: rows are RS -> cols [0,256)=cA (save), [256,512)=cV (detail).
    m in {2,3}: rows are RD -> cH | cD, both detail.
    """
    if m < 2:
        nc.scalar.copy(cAp[:, m, :], bQp[:, 0:256])
        nc.scalar.copy(cAt[:, m, :], bQt[:, 0:256])
        _soft_diff_sum(nc, tpool, acc[:, col:col + 1],
                       bQp[:, 256:512], bQt[:, 256:512], thr, "a")
    else:
        _soft_diff_sum(nc, tpool, acc[:, col:col + 1],
                       bQp[:], bQt[:], thr, "b")


def _wav_level2(nc, tc, wpool, tpool, pspool, w1p, wcp, acc, i, cAp, cAt,
                zcol, zrow):
    """Level-2 DWT on cA [256,256] (stored [128, 2, 256])."""
    rw2p = wpool.tile([P, 2, 256], BF16, tag="rw2p")
    rw2t = wpool.tile([P, 2, 256], BF16, tag="rw2t")
    for m in range(2):
        b2p = pspool.tile([P, 256], F32, tag="ps0")
        b2t = pspool.tile([P, 256], F32, tag="ps1")
        nc.tensor.matmul(b2p[:], zcol[:], zrow[:, 0:256], start=True, stop=False)
        nc.tensor.matmul(b2t[:], zcol[:], zrow[:, 0:256], start=True, stop=False)
        for k in range(2):
            st = False
            mm = nc.tensor.matmul
            mm(b2p[:, 64 * k:64 * k + 64],
               cAp[:, k, 128 * m:128 * m + 128], w1p[:, k, 0:64],
               start=st, stop=False)
            mm(b2p[:, 128 + 64 * k:128 + 64 * k + 64],
               cAp[:, k, 128 * m:128 * m + 128], w1p[:, k, 64:128],
               start=False, stop=k == 1)
            mm(b2t[:, 64 * k:64 * k + 64],
               cAt[:, k, 128 * m:128 * m + 128], w1p[:, k, 0:64],
               start=st, stop=False)
            mm(b2t[:, 128 + 64 * k:128 + 64 * k + 64],
               cAt[:, k, 128 * m:128 * m + 128], w1p[:, k, 64:128],
               start=False, stop=k == 1)
        nc.scalar.copy(rw2p[:, m, :], b2p[:])
        nc.vector.tensor_copy(rw2t[:, m, :], b2t[:])

    cA2p = wpool.tile([P, 128], BF16, tag="cA2p")
    cA2t = wpool.tile([P, 128], BF16, tag="cA2t")
    for m in range(2):
        d2p = pspool.tile([P, 256], F32, tag="ps2")
        d2t = pspool.tile([P, 256], F32, tag="ps3")
        nc.tensor.matmul(d2p[:], zcol[:], zrow[:, 0:256], start=True, stop=False)
        nc.tensor.matmul(d2t[:], zcol[:], zrow[:, 0:256], start=True, stop=False)
        for k in range(2):
            st = False
            mm = nc.tensor.matmul
            mm(d2p[:, 64 * k:64 * k + 64],
               rw2p[:, k, 128 * m:128 * m + 128], wcp[:, k, 0:64],
               start=st, stop=False)
            mm(d2p[:, 128 + 64 * k:128 + 64 * k + 64],
               rw2p[:, k, 128 * m:128 * m + 128], wcp[:, k, 64:128],
               start=False, stop=k == 1)
            mm(d2t[:, 64 * k:64 * k + 64],
               rw2t[:, k, 128 * m:128 * m + 128], wcp[:, k, 0:64],
               start=st, stop=False)
            mm(d2t[:, 128 + 64 * k:128 + 64 * k + 64],
               rw2t[:, k, 128 * m:128 * m + 128], wcp[:, k, 64:128],
               start=False, stop=k == 1)
        col = COL_W2 + 2 * i + m
        if m == 0:
            nc.scalar.copy(cA2p[:], d2p[:, 0:128])
            nc.scalar.copy(cA2t[:], d2t[:, 0:128])
            _soft_diff_sum(nc, tpool, acc[:, col:col + 1],
                           d2p[:, 128:256], d2t[:, 128:256], T_LVL[2], "c")
        else:
            _soft_diff_sum(nc, tpool, acc[:, col:col + 1],
                           d2p[:], d2t[:], T_LVL[2], "d")
    return cA2p, cA2t


def _wav_level3(nc, wpool, tpool, pspool, w1p, wcp, acc, i, cA2p, cA2t,
                zcol, zrow):
    """Level-3 DWT on cA2 [128,128]."""
    rw3p = wpool.tile([P, 128], BF16, tag="rw3p")
    rw3t = wpool.tile([P, 128], BF16, tag="rw3t")
    b3p = pspool.tile([P, 128], F32, tag="ps0")
    b3t = pspool.tile([P, 128], F32, tag="ps1")
    mm = nc.tensor.matmul
    mm(b3p[:], zcol[:], zrow[:, 0:128], start=True, stop=False)
    mm(b3t[:], zcol[:], zrow[:, 0:128], start=True, stop=False)
    mm(b3p[:, 0:64], cA2p[:], w1p[:, 0, 0:64], start=False, stop=False)
    mm(b3p[:, 64:128], cA2p[:], w1p[:, 0, 64:128], start=False, stop=True)
    mm(b3t[:, 0:64], cA2t[:], w1p[:, 0, 0:64], start=False, stop=False)
    mm(b3t[:, 64:128], cA2t[:], w1p[:, 0, 64:128], start=False, stop=True)
    nc.scalar.copy(rw3p[:], b3p[:])
    nc.vector.tensor_copy(rw3t[:], b3t[:])

    d3p = pspool.tile([P, 128], F32, tag="ps2")
    d3t = pspool.tile([P, 128], F32, tag="ps3")
    mm(d3p[:], zcol[:], zrow[:, 0:128], start=True, stop=False)
    mm(d3t[:], zcol[:], zrow[:, 0:128], start=True, stop=False)
    mm(d3p[:, 0:64], rw3p[:], wcp[:, 0, 0:64], start=False, stop=False)
    mm(d3p[:, 64:128], rw3p[:], wcp[:, 0, 64:128], start=False, stop=True)
    mm(d3t[:, 0:64], rw3t[:], wcp[:, 0, 0:64], start=False, stop=False)
    mm(d3t[:, 64:128], rw3t[:], wcp[:, 0, 64:128], start=False, stop=True)
    # quadrants: partitions 0:64 = RS rows (cA3 | cV3), 64:128 = RD (cH3|cD3)
    # detail fields: cV3 = [0:64, 64:128], cH3+cD3 = [64:128, 0:128]
    col = COL_W3 + 2 * i
    _soft_diff_sum(nc, tpool, acc[0:64, col:col + 1],
                   d3p[0:64, 64:128], d3t[0:64, 64:128], T_LVL[3], "e")
    _soft_diff_sum(nc, tpool, acc[64:128, col + 1:col + 2],
                   d3p[64:128, 0:128], d3t[64:128, 0:128], T_LVL[3], "f")


def make_in_maps(pred, target):
    """pred/target: [32, 512, 512] f32 -> list of 8 per-core input dicts."""
    gp, w1p, wcp, gf = _build_consts()
    gp2 = (gp.astype(np.float32) * 2.0).astype(_np_bf16())
    gf2 = (gf.astype(np.float32) * 2.0).astype(_np_bf16())
    maps = []
    for c in range(NCORES):
        maps.append({
            "pred": np.ascontiguousarray(pred[NIMG * c:NIMG * (c + 1)]),
            "target": np.ascontiguousarray(target[NIMG * c:NIMG * (c + 1)]),
            "gp": gp, "gp2": gp2, "gf": gf, "gf2": gf2,
            "w1p": w1p, "wcp": wcp,
        })
    return maps


_NC_CACHE = None


def _get_nc():
    global _NC_CACHE
    if _NC_CACHE is None:
        _NC_CACHE = build_nc()
    return _NC_CACHE


def kernel(pred: np.ndarray, target: np.ndarray) -> np.ndarray:
    from concourse.bass_utils import run_bass_kernel_spmd

    pred = np.ascontiguousarray(np.asarray(pred, dtype=np.float32)
                                .reshape(32, H, W))
    target = np.ascontiguousarray(np.asarray(target, dtype=np.float32)
                                  .reshape(32, H, W))
    in_maps = make_in_maps(pred, target)

    nc = _get_nc()
    res = run_bass_kernel_spmd(nc, in_maps, core_ids=list(range(NCORES)))
    partials = np.stack([r["out"][0].astype(np.float64)
                         for r in res.results])  # [8, 64]
    tot = partials.sum(axis=0)

    npix = 32.0 * H * W
    l1 = tot[COL_L1:COL_L1 + 4].sum() / npix
    ssim_mean = tot[COL_SSIM:COL_SSIM + 16].sum() / npix
    ssim_loss = np.clip(1.0 - ssim_mean, 0.0, 2.0)
    w1 = tot[COL_W1:COL_W1 + 16].sum()   # finest: 256^2 bands
    w2 = tot[COL_W2:COL_W2 + 8].sum()    # 128^2 bands
    w3 = tot[COL_W3:COL_W3 + 8].sum()    # coarsest: 64^2 bands
    wav = (
        (w3 / (32.0 * 64 * 64) / 3.0) / 1.0
        + (w2 / (32.0 * 128 * 128) / 3.0) / 2.0
        + (w1 / (32.0 * 256 * 256) / 3.0) / 3.0
    )
    loss = l1 + 0.5 * ssim_loss + 0.1 * wav
    return np.float32(loss)
